# revision 1
# baseline (speedup 1.0000x reference)
"""Trainium2 Bass kernel for windowed (sparse) cross-attention.

Computation (per batch row b of x:(B=2048, N=64, D=512)):
  q/k/v = x @ Wq/Wk/Wv, split into 8 heads of dim 64.
  13 temporal windows of size 16, stride 4 over N=64; softmax attention within
  each window; overlapping window outputs are accumulated and divided by the
  per-position window count; out = value @ Wout + bout.

Strategy (pure data parallel over 8 NeuronCores, batch-sharded):
  - Host pre-transposes the x shard to xT (D, T) and casts operands to bf16.
  - Projections produce qT/kT (inner-on-partitions) and v (tokens-on-partitions).
  - Scores S'[m,n] = k_m . q_n are computed per (2-batch x 2-head) "quad" with
    K=64 matmuls using tile_position row halves; the full 64x64 score block per
    pair is materialized (windows are diagonal 16x16 sub-blocks of it).
  - Window softmax is linear-algebra-ified: with E = exp(S*scale),
      window sums   s[w, n] = (MaskStack^T @ E)        (one matmul)
      R'[m, n] = MaskStack @ (U * 1/s)                 (one matmul)
      P'[m, n] = E * R'                                (elementwise)
    where U[w,n] = 1[n in window w] / cnt[n].  Then value^T = v^T-contracted
    AV matmuls over P' columns.  This makes the entire softmax+window-overlap
    accumulation 2 small matmuls + 3 vector ops per 2-quad unit.
  - Output projection consumes value^T chunks as stationary operands and
    produces the output in natural (token, D) layout; bias added on DVE.
"""

import sys

if "/opt/trn_rl_repo" not in sys.path:
    sys.path.insert(0, "/opt/trn_rl_repo")

import numpy as np
import ml_dtypes

import concourse.bass as bass
import concourse.tile as tile
from concourse import mybir
from concourse.bass_utils import run_bass_kernel_spmd

BF16 = mybir.dt.bfloat16
F32 = mybir.dt.float32
NP_BF16 = ml_dtypes.bfloat16

# Problem constants (hardcoded per contract)
B, N, D = 2048, 64, 512
NCORES = 8
BC = B // NCORES          # batch rows per core
T_FULL = BC * N           # tokens per core = 16384
HEADS, DH = 8, 64
WINDOW, STRIDE, NW = 16, 4, 13
SCALE = DH ** -0.5
TB = 512                  # tokens per block (8 batch rows)

# stash for test harness introspection
last_results = None


def _split_waits(nc, keep=1):
    """walrus in this toolchain supports only one embedded sync wait per
    instruction; hoist excess waits onto standalone EventSemaphore
    instructions on the same engine queue (FIFO => executes first)."""
    ctr = 0
    for f in nc.m.functions:
        for blk in f.blocks:
            il = blk.instructions
            out = []
            changed = False
            for inst in il:
                si = inst.sync_info
                if si is not None and len(si.on_wait) > keep:
                    waits = list(si.on_wait)
                    SyncInfo = type(si)
                    for w in waits[:-keep]:
                        evs = mybir.InstEventSemaphore(
                            name=f"WSPLIT-{ctr}", ins=[], outs=[]
                        )
                        ctr += 1
                        evs.engine = inst.engine
                        evs.sync_info = SyncInfo(on_wait=[w], on_update=[])
                        out.append(evs)
                    inst.sync_info = SyncInfo(
                        on_wait=waits[-keep:], on_update=list(si.on_update)
                    )
                    changed = True
                out.append(inst)
            if changed:
                il[:] = out
    return ctr


def _window_consts():
    idx = np.arange(NW)[:, None] * STRIDE + np.arange(WINDOW)[None, :]
    cnt = np.zeros(N, dtype=np.float64)
    np.add.at(cnt, idx, 1.0)
    member = np.zeros((N, NW), dtype=np.float64)  # member[m, w] = m in window w
    for w in range(NW):
        member[idx[w], w] = 1.0
    mask_s = np.zeros((128, 26), dtype=np.float64)
    mask_s[:64, :13] = member
    mask_s[64:, 13:] = member
    mask_t = mask_s.T.copy()
    u = np.zeros((26, 512), dtype=np.float64)
    for j in range(512):
        s = ((j % 256) // 64) % 2
        n = j % 64
        u[s * 13:(s + 1) * 13, j] = member[n] / cnt[n]
    return (
        mask_s.astype(NP_BF16),
        mask_t.astype(NP_BF16),
        u.astype(np.float32),
    )


def build_program(T=T_FULL):
    nc = bass.Bass()
    xt_d = nc.dram_tensor("xt", [D, T], BF16, kind="ExternalInput")
    wq_d = nc.dram_tensor("wq", [128, 4, D], BF16, kind="ExternalInput")
    wk_d = nc.dram_tensor("wk", [128, 4, D], BF16, kind="ExternalInput")
    wv_d = nc.dram_tensor("wv", [128, 4, D], BF16, kind="ExternalInput")
    wo_d = nc.dram_tensor("wo", [128, 4, D], BF16, kind="ExternalInput")
    bo_d = nc.dram_tensor("bo", [128, D], F32, kind="ExternalInput")
    ms_d = nc.dram_tensor("ms", [128, 26], BF16, kind="ExternalInput")
    mt_d = nc.dram_tensor("mt", [26, 128], BF16, kind="ExternalInput")
    u_d = nc.dram_tensor("u", [26, 512], F32, kind="ExternalInput")
    out_d = nc.dram_tensor("out", [T, D], F32, kind="ExternalOutput")

    NB = T // TB
    EXP = mybir.ActivationFunctionType.Exp

    with tile.TileContext(nc) as tc:
        with (
            tc.tile_pool(name="consts", bufs=1) as consts,
            tc.tile_pool(name="xtp", bufs=8) as xt_pool,
            tc.tile_pool(name="qkp", bufs=16) as qk_pool,
            tc.tile_pool(name="vp", bufs=8) as v_pool,
            tc.tile_pool(name="ep", bufs=4) as e_pool,
            tc.tile_pool(name="rcp", bufs=4) as rc_pool,
            tc.tile_pool(name="pp", bufs=4) as p_pool,
            tc.tile_pool(name="vtp", bufs=8) as vt_pool,
            tc.tile_pool(name="op", bufs=4) as out_pool,
            tc.tile_pool(name="ps_proj", bufs=2, space="PSUM") as ps_proj,
            tc.tile_pool(name="ps_s", bufs=2, space="PSUM") as ps_s,
            tc.tile_pool(name="ps_w", bufs=1, space="PSUM") as ps_w,
            tc.tile_pool(name="ps_r", bufs=1, space="PSUM") as ps_r,
            tc.tile_pool(name="ps_av", bufs=2, space="PSUM") as ps_av,
        ):
            wq_t = consts.tile([128, 4, D], BF16, tag="wq")
            nc.sync.dma_start(wq_t[:], wq_d[:])
            wk_t = consts.tile([128, 4, D], BF16, tag="wk")
            nc.sync.dma_start(wk_t[:], wk_d[:])
            wv_t = consts.tile([128, 4, D], BF16, tag="wv")
            nc.sync.dma_start(wv_t[:], wv_d[:])
            wo_t = consts.tile([128, 4, D], BF16, tag="wo")
            nc.sync.dma_start(wo_t[:], wo_d[:])
            bo_t = consts.tile([128, D], F32, tag="bo")
            nc.sync.dma_start(bo_t[:], bo_d[:])
            ms_t = consts.tile([128, 26], BF16, tag="ms")
            nc.sync.dma_start(ms_t[:], ms_d[:])
            mt_t = consts.tile([26, 128], BF16, tag="mt")
            nc.sync.dma_start(mt_t[:], mt_d[:])
            u_t = consts.tile([26, 512], F32, tag="u")
            nc.sync.dma_start(u_t[:], u_d[:])

            for blk in range(NB):
                t0 = blk * TB

                # ---- load xT tiles (D on partitions, 4 chunks) ----
                xts = []
                for kc in range(4):
                    xt_t = xt_pool.tile([128, TB], BF16, tag="xt")
                    nc.sync.dma_start(
                        xt_t[:], xt_d[kc * 128:(kc + 1) * 128, t0:t0 + TB]
                    )
                    xts.append(xt_t)

                # ---- qT / kT projections, stored as per-head-half tiles
                # [64, TB] at base partition 0 (avoids partition-offset
                # matmul operands, which wedge this hardware) ----
                qts, kts = [], []
                for wt, lst in ((wq_t, qts), (wk_t, kts)):
                    for c in range(4):
                        ps = ps_proj.tile([128, TB], F32, tag="pp")
                        for kc in range(4):
                            nc.tensor.matmul(
                                ps[:],
                                wt[:, kc, c * 128:(c + 1) * 128],
                                xts[kc][:],
                                start=(kc == 0),
                                stop=(kc == 3),
                            )
                        halves = []
                        for hh in range(2):
                            sb = qk_pool.tile([64, TB], BF16, tag="qk")
                            nc.scalar.copy(sb[:], ps[hh * 64:(hh + 1) * 64, :])
                            halves.append(sb)
                        lst.append(halves)

                # ---- v projection: natural layout [128 tokens, 512 i] ----
                vts = []
                for tt in range(4):
                    ps = ps_proj.tile([128, 512], F32, tag="pp")
                    for kc in range(4):
                        nc.tensor.matmul(
                            ps[:],
                            xts[kc][:, tt * 128:(tt + 1) * 128],
                            wv_t[:, kc, :],
                            start=(kc == 0),
                            stop=(kc == 3),
                        )
                    sb = v_pool.tile([128, 512], BF16, tag="vv")
                    nc.vector.tensor_copy(sb[:], ps[:])
                    vts.append(sb)

                # ---- attention per chunk (2 heads) ----
                vt_out = []
                for c in range(4):
                    qc, kc_t = qts[c], kts[c]
                    av = ps_av.tile([128, 512], F32, tag="av")
                    for tb2 in range(2):
                        # unit: 2 quads (each quad = 2 batch rows x 2 heads)
                        sp = ps_s.tile([128, 512], F32, tag="sp")
                        for qd in range(2):
                            tb = tb2 * 2 + qd
                            for hh in range(2):
                                tcols = slice(tb * 128, (tb + 1) * 128)
                                o = sp[:, qd * 256 + hh * 128:
                                       qd * 256 + (hh + 1) * 128]
                                nc.tensor.matmul(
                                    o, kc_t[hh][:, tcols], qc[hh][:, tcols],
                                    start=True, stop=True,
                                )
                        eu = e_pool.tile([128, 512], BF16, tag="eu")
                        nc.scalar.activation(eu[:], sp[:], EXP, scale=float(SCALE))
                        # window sums for all 4 pairs: [26, 512]
                        sw = ps_w.tile([128, 512], F32, tag="sw")
                        nc.tensor.matmul(sw[:26, :], ms_t[:], eu[:], start=True, stop=True)
                        rc = rc_pool.tile([26, 512], F32, tag="rc")
                        nc.vector.reciprocal(rc[:], sw[:26, :])
                        rcu = rc_pool.tile([26, 512], BF16, tag="rcu")
                        nc.vector.tensor_mul(rcu[:], rc[:], u_t[:])
                        rp = ps_r.tile([128, 512], F32, tag="rp")
                        nc.tensor.matmul(rp[:], mt_t[:], rcu[:], start=True, stop=True)
                        pu = p_pool.tile([128, 512], BF16, tag="pu")
                        nc.vector.tensor_mul(pu[:], eu[:], rp[:])
                        # AV: value^T quad blocks -> av[:, tb*128 + ...]
                        for qd in range(2):
                            tb = tb2 * 2 + qd
                            for hh in range(2):
                                lhsT = vts[tb][
                                    :, c * 128 + hh * 64: c * 128 + hh * 64 + 64
                                ]
                                rhs = pu[:, qd * 256 + hh * 128:
                                         qd * 256 + (hh + 1) * 128]
                                o = av[hh * 64:(hh + 1) * 64,
                                       tb * 128:(tb + 1) * 128]
                                nc.tensor.matmul(o, lhsT, rhs, start=True, stop=True)
                    vt = vt_pool.tile([128, 512], BF16, tag="vt")
                    nc.scalar.copy(vt[:], av[:])
                    vt_out.append(vt)

                # ---- output projection + bias ----
                for tt in range(4):
                    ps = ps_proj.tile([128, 512], F32, tag="pp")
                    for c in range(4):
                        nc.tensor.matmul(
                            ps[:],
                            vt_out[c][:, tt * 128:(tt + 1) * 128],
                            wo_t[:, c, :],
                            start=(c == 0),
                            stop=(c == 3),
                        )
                    ob = out_pool.tile([128, 512], F32, tag="ob")
                    nc.vector.tensor_add(ob[:], ps[:], bo_t[:])
                    nc.sync.dma_start(
                        out_d[t0 + tt * 128: t0 + (tt + 1) * 128, :], ob[:]
                    )
    return nc


def _prep_shared(Wq, Wk, Wv, Wout, bout):
    def warr(w):
        return np.ascontiguousarray(
            w.astype(np.float32).reshape(4, 128, D).transpose(1, 0, 2)
        ).astype(NP_BF16)

    mask_s, mask_t, u = _window_consts()
    return {
        "wq": warr(Wq),
        "wk": warr(Wk),
        "wv": warr(Wv),
        "wo": warr(Wout),
        "bo": np.ascontiguousarray(
            np.broadcast_to(bout.astype(np.float32), (128, D))
        ),
        "ms": mask_s,
        "mt": mask_t,
        "u": u,
    }


def kernel(x, Wq, Wk, Wv, Wout, bout):
    global last_results
    x = np.asarray(x, dtype=np.float32)
    shared = _prep_shared(
        np.asarray(Wq), np.asarray(Wk), np.asarray(Wv),
        np.asarray(Wout), np.asarray(bout),
    )
    in_maps = []
    for ci in range(NCORES):
        xs = x[ci * BC:(ci + 1) * BC].reshape(T_FULL, D)
        xt = np.ascontiguousarray(xs.T).astype(NP_BF16)
        in_maps.append({"xt": xt, **shared})

    nc = build_program(T_FULL)
    _split_waits(nc)
    res = run_bass_kernel_spmd(nc, in_maps, list(range(NCORES)))
    last_results = res
    outs = [
        res.results[ci]["out"].astype(np.float32).reshape(BC, N, D)
        for ci in range(NCORES)
    ]
    return np.concatenate(outs, axis=0)



# revision 12
# speedup vs baseline: 2.1556x; 2.1556x over previous
"""Trainium2 Bass kernel for windowed (sparse) cross-attention.

Computation (per batch row b of x:(B=2048, N=64, D=512)):
  q/k/v = x @ Wq/Wk/Wv, split into 8 heads of dim 64.
  13 temporal windows of size 16, stride 4 over N=64; softmax attention within
  each window; overlapping window outputs are accumulated and divided by the
  per-position window count; out = value @ Wout + bout.

Strategy (pure data parallel over 8 NeuronCores, batch-sharded):
  - Host pre-transposes the x shard to xT (D, T) and casts operands to bf16.
  - Projections produce qT/kT (inner-on-partitions) and v (tokens-on-partitions).
  - Scores S'[m,n] = k_m . q_n are computed per (2-batch x 2-head) "quad"; the
    full 64x64 score block per pair is materialized (windows are diagonal
    16x16 sub-blocks of it).
  - Window softmax is linear-algebra-ified: with E = exp(S*scale),
      window sums   s[w, n] = (MaskStack^T @ E)        (one matmul)
      R'[m, n] = MaskStack @ (U * 1/s)                 (one matmul)
      P'[m, n] = E * R'                                (elementwise)
    where U[w,n] = 1[n in window w] / cnt[n].  Then value^T = v^T-contracted
    AV matmuls over P' columns.
  - 1/s = exp(-ln(s)) on the Activation engine (ln and exp live in one act
    table); the stock DVE InstReciprocal is ~3.3us per [26,512] tile and was
    the baseline's top hotspot.
  - 1/cnt and bout are folded into the out-projection: the PSUM is initialized
    with cnt[row]*bout[col] via an init matmul cnt[1,128]^T @ bout[1,512], and
    the final PSUM->SBUF copy is an Identity activation with per-partition
    scale 1/cnt[row].  U then becomes an exact 0/1 membership mask.
  - Emission is software-pipelined globally: attention of block N-1 is
    interleaved round-by-round with the projections of block N so the PE never
    head-blocks on the softmax chain, and elementwise work is split ~evenly
    between the Activation and Vector engines.
"""

import sys

if "/opt/trn_rl_repo" not in sys.path:
    sys.path.insert(0, "/opt/trn_rl_repo")

import numpy as np
import ml_dtypes

import concourse.bass as bass
import concourse.tile as tile
from concourse import mybir
from concourse.bass_utils import run_bass_kernel_spmd

BF16 = mybir.dt.bfloat16
F32 = mybir.dt.float32
NP_BF16 = ml_dtypes.bfloat16

# Problem constants (hardcoded per contract)
B, N, D = 2048, 64, 512
NCORES = 8
BC = B // NCORES          # batch rows per core
T_FULL = BC * N           # tokens per core = 16384
HEADS, DH = 8, 64
WINDOW, STRIDE, NW = 16, 4, 13
SCALE = DH ** -0.5
TB = 512                  # tokens per block (8 batch rows)

# stash for test harness introspection
last_results = None


def _split_waits(nc, keep=1):
    """walrus in this toolchain supports only one embedded sync wait per
    instruction; hoist excess waits onto standalone EventSemaphore
    instructions on the same engine queue (FIFO => executes first)."""
    ctr = 0
    for f in nc.m.functions:
        for blk in f.blocks:
            il = blk.instructions
            out = []
            changed = False
            for inst in il:
                si = inst.sync_info
                if si is not None and len(si.on_wait) > keep:
                    waits = list(si.on_wait)
                    SyncInfo = type(si)
                    for w in waits[:-keep]:
                        evs = mybir.InstEventSemaphore(
                            name=f"WSPLIT-{ctr}", ins=[], outs=[]
                        )
                        ctr += 1
                        evs.engine = inst.engine
                        evs.sync_info = SyncInfo(on_wait=[w], on_update=[])
                        out.append(evs)
                    inst.sync_info = SyncInfo(
                        on_wait=waits[-keep:], on_update=list(si.on_update)
                    )
                    changed = True
                out.append(inst)
            if changed:
                il[:] = out
    return ctr


def _window_consts():
    idx = np.arange(NW)[:, None] * STRIDE + np.arange(WINDOW)[None, :]
    cnt = np.zeros(N, dtype=np.float64)
    np.add.at(cnt, idx, 1.0)
    member = np.zeros((N, NW), dtype=np.float64)  # member[m, w] = m in window w
    for w in range(NW):
        member[idx[w], w] = 1.0
    mask_s = np.zeros((128, 26), dtype=np.float64)
    mask_s[:64, :13] = member
    mask_s[64:, 13:] = member
    mask_t = mask_s.T.copy()
    u = np.zeros((26, 512), dtype=np.float64)
    for j in range(512):
        s = ((j % 256) // 64) % 2
        n = j % 64
        u[s * 13:(s + 1) * 13, j] = member[n]  # 1/cnt folded into out-proj
    # per-token window counts for a 128-token tile (pattern repeats mod 64)
    cnt_row = np.tile(cnt, 2)[None, :]          # [1, 128]
    cnt_inv = (1.0 / np.tile(cnt, 2))[:, None]  # [128, 1]
    return (
        mask_s.astype(NP_BF16),
        mask_t.astype(NP_BF16),
        u.astype(NP_BF16),
        cnt_row.astype(NP_BF16),
        cnt_inv.astype(np.float32),
    )


def build_program(T=T_FULL):
    nc = bass.Bass()
    xt_d = nc.dram_tensor("xt", [D, T], BF16, kind="ExternalInput")
    wq_d = nc.dram_tensor("wq", [128, 4, D], BF16, kind="ExternalInput")
    wk_d = nc.dram_tensor("wk", [128, 4, D], BF16, kind="ExternalInput")
    wv_d = nc.dram_tensor("wv", [128, 4, D], BF16, kind="ExternalInput")
    wo_d = nc.dram_tensor("wo", [128, 4, D], BF16, kind="ExternalInput")
    bo_d = nc.dram_tensor("bo", [1, D], BF16, kind="ExternalInput")
    cr_d = nc.dram_tensor("cr", [1, 128], BF16, kind="ExternalInput")
    ci_d = nc.dram_tensor("ci", [128, 1], F32, kind="ExternalInput")
    ms_d = nc.dram_tensor("ms", [128, 26], BF16, kind="ExternalInput")
    mt_d = nc.dram_tensor("mt", [26, 128], BF16, kind="ExternalInput")
    u_d = nc.dram_tensor("u", [26, 512], BF16, kind="ExternalInput")
    out_d = nc.dram_tensor("out", [T, D], F32, kind="ExternalOutput")

    NB = T // TB
    NU = NB * 8  # global unit count (8 softmax units per block)
    EXP = mybir.ActivationFunctionType.Exp
    LN = mybir.ActivationFunctionType.Ln
    IDN = mybir.ActivationFunctionType.Identity

    with tile.TileContext(nc) as tc:
        with (
            tc.tile_pool(name="consts", bufs=1) as consts,
            tc.tile_pool(name="xtp", bufs=12) as xt_pool,
            tc.tile_pool(name="qkp", bufs=32) as qk_pool,
            tc.tile_pool(name="eup", bufs=6) as eu_pool,
            tc.tile_pool(name="rcp", bufs=4) as rc_pool,
            tc.tile_pool(name="rcup", bufs=4) as rcu_pool,
            tc.tile_pool(name="pup", bufs=4) as pu_pool,
            tc.tile_pool(name="vp", bufs=12) as v_pool,
            tc.tile_pool(name="vtp", bufs=12) as vt_pool,
            tc.tile_pool(name="osp", bufs=6) as os_pool,
            tc.tile_pool(name="ps_proj", bufs=3, space="PSUM") as ps_proj,
            tc.tile_pool(name="ps_s", bufs=2, space="PSUM") as ps_s,
            tc.tile_pool(name="ps_sw", bufs=1, space="PSUM") as ps_sw,
            tc.tile_pool(name="ps_r", bufs=1, space="PSUM") as ps_r,
            tc.tile_pool(name="ps_av", bufs=1, space="PSUM") as ps_av,
        ):
            wq_t = consts.tile([128, 4, D], BF16, tag="wq")
            nc.sync.dma_start(wq_t[:], wq_d[:])
            wk_t = consts.tile([128, 4, D], BF16, tag="wk")
            nc.sync.dma_start(wk_t[:], wk_d[:])
            wv_t = consts.tile([128, 4, D], BF16, tag="wv")
            nc.sync.dma_start(wv_t[:], wv_d[:])
            wo_t = consts.tile([128, 4, D], BF16, tag="wo")
            nc.sync.dma_start(wo_t[:], wo_d[:])
            bo_t = consts.tile([1, D], BF16, tag="bo")
            nc.sync.dma_start(bo_t[:], bo_d[:])
            cr_t = consts.tile([1, 128], BF16, tag="cr")
            nc.sync.dma_start(cr_t[:], cr_d[:])
            ci_t = consts.tile([128, 1], F32, tag="ci")
            nc.sync.dma_start(ci_t[:], ci_d[:])
            ms_t = consts.tile([128, 26], BF16, tag="ms")
            nc.sync.dma_start(ms_t[:], ms_d[:])
            mt_t = consts.tile([26, 128], BF16, tag="mt")
            nc.sync.dma_start(mt_t[:], mt_d[:])
            u_t = consts.tile([26, 512], BF16, tag="u")
            nc.sync.dma_start(u_t[:], u_d[:])

            # one PSUM bank, manually rotated through 4 sub-slots at 32-aligned
            # partition offsets (subtile deps keep the rotation correct)
            sw_bank = ps_sw.tile([128, 512], F32, tag="sw")

            xts = {}        # (blk, kc) -> xT tile
            qk_tiles = {}   # (blk, wi, c) -> [half0, half1]
            v_tiles = {}    # (blk, tt)
            vt_tiles = {}   # (blk, c)
            eu_tiles = {}   # g
            rcu_tiles = {}  # g
            pu_tiles = {}   # g
            av_tiles = {}   # (blk, c)
            proj_cache = {}

            def dma_xt(b):
                for kc in range(4):
                    xt_t = xt_pool.tile([128, TB], BF16, tag="xt", name=f"xt_{b}_{kc}")
                    nc.sync.dma_start(
                        xt_t[:], xt_d[kc * 128:(kc + 1) * 128, b * TB:(b + 1) * TB]
                    )
                    xts[(b, kc)] = xt_t

            copy_ctr = [0]

            def _balanced_copy(dst, src):
                # ACT carries exp/ln/rexp/out (~16us/blk); DVE carries
                # rcu/pu/vt (~11us/blk). Send ~5/20 proj copies to ACT.
                i = copy_ctr[0] % 4
                copy_ctr[0] += 1
                if i == 3:
                    nc.scalar.copy(dst, src)
                else:
                    nc.vector.tensor_copy(dst, src)

            def qk_group(b, wi, wt, c):
                ps = ps_proj.tile([128, TB], F32, tag="pp", name=f"pq_{b}_{wi}_{c}")
                for kc in range(4):
                    nc.tensor.matmul(
                        ps[:],
                        wt[:, kc, c * 128:(c + 1) * 128],
                        xts[(b, kc)][:],
                        start=(kc == 0),
                        stop=(kc == 3),
                    )
                halves = []
                for hh in range(2):
                    sb = qk_pool.tile([64, TB], BF16, tag="qk", name=f"qk_{b}_{wi}_{c}_{hh}")
                    _balanced_copy(sb[:], ps[hh * 64:(hh + 1) * 64, :])
                    halves.append(sb)
                qk_tiles[(b, wi, c)] = halves

            def v_group(b, tt):
                ps = ps_proj.tile([128, 512], F32, tag="pp", name=f"pv_{b}_{tt}")
                for kc in range(4):
                    nc.tensor.matmul(
                        ps[:],
                        xts[(b, kc)][:, tt * 128:(tt + 1) * 128],
                        wv_t[:, kc, :],
                        start=(kc == 0),
                        stop=(kc == 3),
                    )
                sb = v_pool.tile([128, 512], BF16, tag="vv", name=f"v_{b}_{tt}")
                _balanced_copy(sb[:], ps[:])
                v_tiles[(b, tt)] = sb

            def proj_groups(b):
                groups = []
                for wi, wt in ((0, wq_t), (1, wk_t)):
                    for c in range(4):
                        groups.append((qk_group, b, wi, wt, c))
                for tt in range(4):
                    groups.append((v_group, b, tt))
                return groups

            def out_group(b, tt):
                ps = ps_proj.tile([128, 512], F32, tag="pp", name=f"po_{b}_{tt}")
                # PSUM init = cnt[row] * bout[col]; final copy scales by
                # 1/cnt[row], yielding value/cnt @ Wout + bout in one pass.
                nc.tensor.matmul(ps[:], cr_t[:], bo_t[:], start=True, stop=False)
                for c in range(4):
                    nc.tensor.matmul(
                        ps[:],
                        vt_tiles[(b, c)][:, tt * 128:(tt + 1) * 128],
                        wo_t[:, c, :],
                        start=False,
                        stop=(c == 3),
                    )
                ob = os_pool.tile([128, 512], F32, tag="ob", name=f"ob_{b}_{tt}")
                nc.scalar.activation(ob[:], ps[:], IDN, scale=ci_t[:, 0:1])
                nc.sync.dma_start(
                    out_d[b * TB + tt * 128: b * TB + (tt + 1) * 128, :], ob[:]
                )

            def stage_sc(g):
                b, u = divmod(g, 8)
                c, tb2 = divmod(u, 2)
                qc = qk_tiles[(b, 0, c)]
                kc_t = qk_tiles[(b, 1, c)]
                sp = ps_s.tile([128, 512], F32, tag="sp", name=f"sp_{g}")
                for qd in range(2):
                    tb = tb2 * 2 + qd
                    for hh in range(2):
                        tcols = slice(tb * 128, (tb + 1) * 128)
                        o = sp[:, qd * 256 + hh * 128: qd * 256 + (hh + 1) * 128]
                        nc.tensor.matmul(
                            o, kc_t[hh][:, tcols], qc[hh][:, tcols],
                            start=True, stop=True,
                        )
                eu = eu_pool.tile([128, 512], BF16, tag="eu", name=f"eu_{g}")
                nc.scalar.activation(eu[:], sp[:], EXP, scale=float(SCALE))
                eu_tiles[g] = eu
                if (b, 0, c) != (g // 8, 0, (g % 8) // 2):
                    raise AssertionError
                # release qk refs once the last unit of chunk c used them
                if tb2 == 1:
                    del qk_tiles[(b, 0, c)]
                    del qk_tiles[(b, 1, c)]

            def stage_ms(g):
                off = 32 * (g % 3)  # matmul out base partition must be 0/32/64
                swv = sw_bank[off:off + 26, :]
                nc.tensor.matmul(swv, ms_t[:], eu_tiles[g][:], start=True, stop=True)
                # 1/s = exp(-ln(s)) on ACT (DVE reciprocal is ~8 cyc/elem)
                lg = rc_pool.tile([26, 512], F32, tag="rc", name=f"lg_{g}")
                nc.scalar.activation(lg[:], swv, LN)
                rc = rc_pool.tile([26, 512], BF16, tag="rcb", name=f"rc_{g}")
                nc.scalar.activation(rc[:], lg[:], EXP, scale=-1.0)
                rcu = rcu_pool.tile([26, 512], BF16, tag="rcu", name=f"rcu_{g}")
                nc.vector.tensor_mul(rcu[:], rc[:], u_t[:])
                rcu_tiles[g] = rcu

            def stage_mt(g):
                rp = ps_r.tile([128, 512], F32, tag="rp", name=f"rp_{g}")
                nc.tensor.matmul(rp[:], mt_t[:], rcu_tiles[g][:], start=True, stop=True)
                pu = pu_pool.tile([128, 512], BF16, tag="pu", name=f"pu_{g}")
                nc.vector.tensor_mul(pu[:], eu_tiles[g][:], rp[:])
                pu_tiles[g] = pu
                del rcu_tiles[g]

            def stage_av(g):
                b, u = divmod(g, 8)
                c, tb2 = divmod(u, 2)
                if tb2 == 0:
                    av_tiles[(b, c)] = ps_av.tile(
                        [128, 512], F32, tag="av", name=f"av_{b}_{c}"
                    )
                av = av_tiles[(b, c)]
                pu = pu_tiles[g]
                for qd in range(2):
                    tb = tb2 * 2 + qd
                    for hh in range(2):
                        lhsT = v_tiles[(b, tb)][
                            :, c * 128 + hh * 64: c * 128 + hh * 64 + 64
                        ]
                        rhs = pu[:, qd * 256 + hh * 128: qd * 256 + (hh + 1) * 128]
                        o = av[hh * 64:(hh + 1) * 64, tb * 128:(tb + 1) * 128]
                        nc.tensor.matmul(o, lhsT, rhs, start=True, stop=True)
                del pu_tiles[g]
                del eu_tiles[g]
                if tb2 == 1:
                    vt = vt_pool.tile([128, 512], BF16, tag="vt", name=f"vt_{b}_{c}")
                    nc.vector.tensor_copy(vt[:], av[:])
                    vt_tiles[(b, c)] = vt
                    del av_tiles[(b, c)]

            # ---- prologue ----
            dma_xt(0)
            dma_xt(1)
            for fn, *args in proj_groups(0):
                fn(*args)

            # ---- main software-pipelined rounds ----
            for g in range(NU + 3):
                b, u = divmod(g, 8)
                if u == 0 and b + 2 < NB:
                    dma_xt(b + 2)
                # stream A: projections of block b+1, paced over the 8 rounds
                if b + 1 < NB:
                    if b + 1 not in proj_cache:
                        proj_cache[b + 1] = proj_groups(b + 1)
                    gl = proj_cache[b + 1]
                    for fn, *args in gl[u * 12 // 8:(u + 1) * 12 // 8]:
                        fn(*args)
                # stream B: attention stages (software pipeline, lag 0/1/2/3)
                if g < NU:
                    stage_sc(g)
                if 0 <= g - 1 < NU:
                    stage_ms(g - 1)
                if 0 <= g - 2 < NU:
                    stage_mt(g - 2)
                if 0 <= g - 3 < NU:
                    stage_av(g - 3)
                # stream C: out-projection of block b-1 (vt tiles complete by u>=3)
                if b - 1 >= 0 and 3 <= u <= 6:
                    out_group(b - 1, u - 3)
            # epilogue: out-projection of the last block
            for tt in range(4):
                out_group(NB - 1, tt)
    return nc


def _prep_shared(Wq, Wk, Wv, Wout, bout):
    def warr(w):
        return np.ascontiguousarray(
            w.astype(np.float32).reshape(4, 128, D).transpose(1, 0, 2)
        ).astype(NP_BF16)

    mask_s, mask_t, u, cnt_row, cnt_inv = _window_consts()
    return {
        "wq": warr(Wq),
        "wk": warr(Wk),
        "wv": warr(Wv),
        "wo": warr(Wout),
        "bo": np.ascontiguousarray(bout.astype(np.float32)[None, :]).astype(NP_BF16),
        "cr": cnt_row,
        "ci": cnt_inv,
        "ms": mask_s,
        "mt": mask_t,
        "u": u,
    }


def kernel(x, Wq, Wk, Wv, Wout, bout):
    global last_results
    x = np.asarray(x, dtype=np.float32)
    shared = _prep_shared(
        np.asarray(Wq), np.asarray(Wk), np.asarray(Wv),
        np.asarray(Wout), np.asarray(bout),
    )
    in_maps = []
    for ci in range(NCORES):
        xs = x[ci * BC:(ci + 1) * BC].reshape(T_FULL, D)
        xt = np.ascontiguousarray(xs.T).astype(NP_BF16)
        in_maps.append({"xt": xt, **shared})

    nc = build_program(T_FULL)
    _split_waits(nc)
    res = run_bass_kernel_spmd(nc, in_maps, list(range(NCORES)))
    last_results = res
    outs = [
        res.results[ci]["out"].astype(np.float32).reshape(BC, N, D)
        for ci in range(NCORES)
    ]
    return np.concatenate(outs, axis=0)
